# revision 4
# baseline (speedup 1.0000x reference)
"""MiniMoE Trainium2 kernel — expert-parallel, two-tier precision.

Problem (hardcoded): x [4, 2048, 1024] f32, router_w [1024, 4], router_b [4],
w1/w3 [4, 1024, 4096], w2 [4, 4096, 1024], top-2 of 4 experts, SwiGLU.

Strategy
--------
Host computes the (tiny) router + top-2 dispatch. Core pair (2e, 2e+1) owns
expert e: core 2e computes the F in [0, 2048) half of the SwiGLU FFN, core
2e+1 the F in [2048, 4096) half, over all tokens routed to expert e. The
partial outputs sum to the expert output, and the host scatter-adds them
with the renormalized gate weights.

Per token-expert pair there are two tiers, split by gate weight:
  precise: plain bf16 matmuls (384 PE-cycles/token, LDWEIGHTS fully
      hidden under the 512-col matmuls) — ~4e-3 rel err;
  single: the up-projections run as fp8(e4m3) DoubleRow matmuls
      (256-deep contraction, ~2x bf16 FLOP rate on HW), h stays bf16 and
      the down-projection is bf16 — ~5.5e-2 rel err, ~256 cycles/token.
Each expert's tokens are sorted by gate descending; the top Cp go to the
precise tier, the rest (small gates, error contribution ~g^2) to the
single tier, so the combined rel err lands ~1.45e-2 against the 2e-2
gate while cutting PE time ~15%.

fp8 scales (powers of two, folded into existing instructions): x: 1,
w1: 32 (silu input scale 1/32), w3: 8 (h inherits the 8, removed by the
1/8 scale on the output copy of single-tier blocks).
"""

import numpy as np
import ml_dtypes

import concourse.bass as bass
import concourse.bacc as bacc
import concourse.tile as tile
import concourse.mybir as mybir
from concourse.bass_utils import run_bass_kernel_spmd

B, S, D, F, E, TOPK = 4, 2048, 1024, 4096, 4, 2
N_CORES = 8
FH = F // 2          # F-half handled per core
P = 128              # SBUF partitions
ND = D // P          # 8 d-blocks (4 DoubleRow pairs)
NF = FH // P         # 16 f-blocks per core
FP8 = mybir.dt.float8e4
BF16 = mybir.dt.bfloat16
F32 = mybir.dt.float32
DR = mybir.MatmulPerfMode.DoubleRow
E4 = ml_dtypes.float8_e4m3

S_W1, S_W3 = 32.0, 8.0
PRECISE_FRAC = 0.722   # target precise-tier share of the max expert load

_NC_CACHE: dict[tuple, object] = {}


def _token_blocks(C: int) -> list[tuple[int, int]]:
    """Token blocks of 512; split a short tail across the last two blocks
    (e.g. 512+128 -> 320+320) so matmul N stays large enough to pipeline."""
    sizes = []
    left = C
    while left > 0:
        tb = min(512, left)
        sizes.append(tb)
        left -= tb
    if len(sizes) >= 2 and sizes[-1] < 512:
        pair = sizes[-2] + sizes[-1]
        hi = ((pair // 2 + 63) // 64) * 64
        sizes[-2:] = [hi, pair - hi]
    blocks, t0 = [], 0
    for tb in sizes:
        blocks.append((t0, tb))
        t0 += tb
    return blocks


def _build_nc(Cp: int, Cs: int, repeat: int = 1):
    """SPMD per-core program: Cp precise (bf16) + Cs single (fp8-up) tokens."""
    nc = bacc.Bacc("TRN2", target_bir_lowering=False, debug=False,
                   num_devices=N_CORES)
    xpb = nc.dram_tensor("xpb", [D, Cp], BF16, kind="ExternalInput").ap()
    xs8 = (nc.dram_tensor("xs8", [D, Cs], FP8, kind="ExternalInput").ap()
           if Cs else None)
    w1b = nc.dram_tensor("w1b", [NF, P, ND * P], BF16, kind="ExternalInput").ap()
    w3b = nc.dram_tensor("w3b", [NF, P, ND * P], BF16, kind="ExternalInput").ap()
    w2b = nc.dram_tensor("w2b", [NF, P, D], BF16, kind="ExternalInput").ap()
    w18 = nc.dram_tensor("w18", [NF, P, ND * P], FP8, kind="ExternalInput").ap()
    w38 = nc.dram_tensor("w38", [NF, P, ND * P], FP8, kind="ExternalInput").ap()
    outT = nc.dram_tensor("outT", [D, Cp + Cs], BF16, kind="ExternalOutput").ap()

    with tile.TileContext(nc) as tc:
        with (
            tc.tile_pool(name="wpool", bufs=1) as wpool,
            tc.tile_pool(name="xpool", bufs=2) as xpool,
            tc.tile_pool(name="hpool", bufs=2) as hpool,
            tc.tile_pool(name="tpool", bufs=2) as tpool,
            tc.tile_pool(name="opool", bufs=3) as opool,
            tc.tile_pool(name="ps1", bufs=2, space=bass.MemorySpace.PSUM) as ps1,
            tc.tile_pool(name="ps2", bufs=4, space=bass.MemorySpace.PSUM) as ps2,
        ):
            xpb_r = xpb.rearrange("(n p) c -> p n c", p=P)
            xs8_r = xs8.rearrange("(n p) c -> p n c", p=P) if Cs else None
            pblocks = _token_blocks(Cp)
            sblocks = _token_blocks(Cs) if Cs else []

            # Startup ordering: first psum group needs w1b[0] + x block 0.
            w1b_f, w3b_f, w2b_f, w18_f, w38_f = [], [], [], [], []

            def wtile(dst, src, ft, tag, dt, shape, rearr):
                t = wpool.tile(shape, dt, tag=f"{tag}_{ft}", name="t")
                nc.sync.dma_start(t[:], src[ft].rearrange(rearr, c=P)
                                  if rearr else src[ft])
                dst.append(t)

            wtile(w1b_f, w1b, 0, "w1b", BF16, [P, ND, P], "p (n c) -> p n c")
            t00, TB0 = pblocks[0]
            xb0 = xpool.tile([P, ND, TB0], BF16, tag="xpb", name="xb0")
            nc.sync.dma_start(xb0[:], xpb_r[:, :, t00:t00 + TB0])
            wtile(w3b_f, w3b, 0, "w3b", BF16, [P, ND, P], "p (n c) -> p n c")
            for ft in range(1, NF):
                wtile(w1b_f, w1b, ft, "w1b", BF16, [P, ND, P], "p (n c) -> p n c")
                wtile(w3b_f, w3b, ft, "w3b", BF16, [P, ND, P], "p (n c) -> p n c")
            for ft in range(NF):
                t = wpool.tile([P, D], BF16, tag=f"w2b_{ft}", name="t")
                nc.sync.dma_start(t[:], w2b[ft])
                w2b_f.append(t)
            for ft in range(NF):
                wtile(w18_f, w18, ft, "w18", FP8, [P, ND, P], "p (n c) -> p n c")
                wtile(w38_f, w38, ft, "w38", FP8, [P, ND, P], "p (n c) -> p n c")

            def do_block(t0, TB, cbase, precise, x_t):
                hT = hpool.tile([P, NF, TB], BF16, tag="hT", name="hT")
                for ft in range(NF):
                    p1 = ps1.tile([P, TB], F32, tag="p1", name="p1")
                    p3 = ps1.tile([P, TB], F32, tag="p3", name="p3")
                    if precise:
                        for wt, ps in ((w1b_f[ft], p1), (w3b_f[ft], p3)):
                            for d in range(ND):
                                nc.tensor.matmul(
                                    ps[:], wt[:, d, :], x_t[:, d, :],
                                    start=(d == 0), stop=(d == ND - 1))
                    else:
                        for wt, ps in ((w18_f[ft], p1), (w38_f[ft], p3)):
                            for p in range(ND // 2):
                                nc.tensor.matmul(
                                    ps[:], wt[:, 2 * p:2 * p + 2, :],
                                    x_t[:, 2 * p:2 * p + 2, :],
                                    start=(p == 0), stop=(p == ND // 2 - 1),
                                    perf_mode=DR)
                    sil = tpool.tile([P, TB], F32, tag="sil", name="sil")
                    nc.scalar.activation(
                        sil[:], p1[:], mybir.ActivationFunctionType.Silu,
                        scale=(1.0 if precise else 1.0 / S_W1))
                    nc.vector.tensor_mul(hT[:, ft, :], sil[:], p3[:])

                for db in range(ND):
                    po = ps2.tile([P, TB], F32, tag="po", name="po")
                    for ft in range(NF):
                        nc.tensor.matmul(
                            po[:], w2b_f[ft][:, db * P:(db + 1) * P],
                            hT[:, ft, :], start=(ft == 0), stop=(ft == NF - 1))
                    ot = opool.tile([P, TB], BF16, tag="ot", name="ot")
                    nc.scalar.mul(ot[:], po[:],
                                  1.0 if precise else 1.0 / S_W3)
                    nc.sync.dma_start(
                        outT[db * P:(db + 1) * P, cbase + t0:cbase + t0 + TB],
                        ot[:])

            for _ in range(repeat):
                for bi, (t0, TB) in enumerate(pblocks):
                    if bi == 0 and TB == TB0:
                        x_t = xb0
                    else:
                        x_t = xpool.tile([P, ND, TB], BF16, tag="xpb", name="x_t")
                        nc.sync.dma_start(x_t[:], xpb_r[:, :, t0:t0 + TB])
                    do_block(t0, TB, 0, True, x_t)
                for t0, TB in sblocks:
                    x_t = xpool.tile([P, ND, TB], FP8, tag="xs8", name="x_t")
                    nc.sync.dma_start(x_t[:], xs8_r[:, :, t0:t0 + TB])
                    do_block(t0, TB, Cp, False, x_t)

    nc.compile()
    return nc


def _route(x, router_w, router_b):
    """Host router: top-2 expert ids + renormalized gates (float64 math)."""
    T = x.shape[0] * x.shape[1]
    xf = x.reshape(T, D).astype(np.float64)
    logits = xf @ router_w.astype(np.float64) + router_b.astype(np.float64)
    # stable sort: ties resolve to the lowest expert id, like jax.lax.top_k
    order = np.argsort(-logits, axis=-1, kind="stable")   # [T, E] descending
    top_i = order[:, :TOPK]                        # [T, 2]
    top_l = np.take_along_axis(logits, top_i, axis=-1)
    top_l -= top_l.max(axis=-1, keepdims=True)
    ex = np.exp(top_l)
    gates = ex / ex.sum(axis=-1, keepdims=True)    # [T, 2] renormalized
    return top_i, gates


def _q8(a, s):
    return np.clip(a * np.float32(s), -240, 240).astype(E4)


def _tile_w(w):  # [D, FH] -> [NF, P, ND*P]; chunk ft == SBUF tile ft
    return np.ascontiguousarray(
        w.reshape(ND, P, NF, P).transpose(2, 1, 0, 3).reshape(NF, P, ND * P))


def prepare(x, router_w, router_b, w1, w3, w2):
    """Route on host, tier tokens, build per-core input maps."""
    T = x.shape[0] * x.shape[1]
    xf = np.ascontiguousarray(x.reshape(T, D), dtype=np.float32)
    top_i, gates = _route(x, router_w, router_b)

    idx_per_e, gate_per_e = [], []
    for e in range(E):
        mask = (top_i == e)
        rows = np.nonzero(mask.any(axis=-1))[0]
        g = np.where(mask[rows, 0], gates[rows, 0], gates[rows, 1])
        order = np.argsort(-g, kind="stable")      # gate descending
        idx_per_e.append(rows[order])
        gate_per_e.append(g[order].astype(np.float32))

    Cmax = max(max(len(r) for r in idx_per_e), 1)
    Cp = min(min(len(r) for r in idx_per_e),
             int(round(PRECISE_FRAC * Cmax)))
    Cs = Cmax - Cp

    xq8 = _q8(xf, 1.0)
    xb = xf.astype(ml_dtypes.bfloat16)

    in_maps = []
    for core in range(N_CORES):
        e, half = core // 2, core % 2
        fs = slice(half * FH, (half + 1) * FH)
        rows = idx_per_e[e]
        prec, sing = rows[:Cp], rows[Cp:]

        def xt(q, rws, C, dt):
            g = np.zeros((C, D), dt)
            g[:len(rws)] = q[rws]
            return np.ascontiguousarray(g.T)

        w1e = w1[e, :, fs].astype(np.float32)
        w3e = w3[e, :, fs].astype(np.float32)
        m = {
            "xpb": xt(xb, prec, Cp, ml_dtypes.bfloat16),
            "w1b": _tile_w(w1e.astype(ml_dtypes.bfloat16)),
            "w3b": _tile_w(w3e.astype(ml_dtypes.bfloat16)),
            "w2b": np.ascontiguousarray(
                w2[e, fs, :].astype(ml_dtypes.bfloat16).reshape(NF, P, D)),
            "w18": _tile_w(_q8(w1e, S_W1)),
            "w38": _tile_w(_q8(w3e, S_W3)),
        }
        if Cs:
            m["xs8"] = xt(xq8, sing, Cs, E4)
        in_maps.append(m)
    meta = (T, Cp, idx_per_e, gate_per_e)
    return Cp, Cs, in_maps, meta


def combine(results, meta):
    """Gate-weighted scatter-add of the per-core partial expert outputs."""
    T, Cp, idx_per_e, gate_per_e = meta
    out = np.zeros((T, D), np.float32)
    for e in range(E):
        rows = idx_per_e[e]
        n = len(rows)
        part = (results[2 * e]["outT"].T[:n].astype(np.float32)
                + results[2 * e + 1]["outT"].T[:n].astype(np.float32))
        out[rows] += gate_per_e[e][:, None] * part
    return out.reshape(B, S, D)


def kernel(**inputs):
    x = np.asarray(inputs["x"], np.float32)
    router_w = np.asarray(inputs["router_w"], np.float32)
    router_b = np.asarray(inputs["router_b"], np.float32)
    w1 = np.asarray(inputs["w1"], np.float32)
    w3 = np.asarray(inputs["w3"], np.float32)
    w2 = np.asarray(inputs["w2"], np.float32)

    Cp, Cs, in_maps, meta = prepare(x, router_w, router_b, w1, w3, w2)
    if (Cp, Cs) not in _NC_CACHE:
        _NC_CACHE[(Cp, Cs)] = _build_nc(Cp, Cs)
    nc = _NC_CACHE[(Cp, Cs)]
    res = run_bass_kernel_spmd(nc, in_maps, list(range(N_CORES)))
    return combine(res.results, meta)
